# revision 2
# baseline (speedup 1.0000x reference)
"""KNN (k=16, 10 classes) on 8 Trainium2 NeuronCores via Bass/Tile.

Strategy (classic distributed ANN):
  - Host: sort X_train by label; shard N_train contiguously across 8 cores;
    pad each per-core class segment to a 512 multiple so every 512-wide
    matmul chunk is class-pure (label == per-chunk constant).
  - Device (per core, SPMD): v[q, j] = 2<t_q, x_j> - ||x_j||^2 computed by
    PE as two accumulating matmuls into PSUM [128q x 512n] (main K=128 plus
    a K=1 ones x (-xx) accumulate).  Ordering by v == ordering by
    -euclidean distance.  DVE max8 reads each PSUM chunk directly ->
    per-chunk top-8 values; a small stage-2 (max8/max_index/match_replace)
    reduces C*8 candidates to the core-local top-16 (values + positions).
  - Host: merge 8 cores x 16 candidates per query by value, map candidate
    position -> chunk -> label, majority vote (argmax ties -> smallest
    label, matching the reference).
"""

import numpy as np

NCORES = 8
CHUNK = 512
K = 16
NUM_CLASSES = 10
QTILE = 128

_compiled_cache = {}


def _build_program(D, NQ, Lp):
    import concourse.bacc as bacc
    import concourse.tile as tile
    import concourse.mybir as mybir

    C = Lp // CHUNK
    nqt = NQ // QTILE

    nc = bacc.Bacc("TRN2", target_bir_lowering=False, debug=False)
    xt = nc.dram_tensor("xt", [D, NQ], mybir.dt.float32, kind="ExternalInput")
    xn = nc.dram_tensor("xn", [D, Lp], mybir.dt.float32, kind="ExternalInput")
    nxx = nc.dram_tensor("nxx", [1, Lp], mybir.dt.float32, kind="ExternalInput")
    out_val = nc.dram_tensor("out_val", [NQ, K], mybir.dt.float32, kind="ExternalOutput")
    out_idx = nc.dram_tensor("out_idx", [NQ, K], mybir.dt.uint32, kind="ExternalOutput")

    with tile.TileContext(nc) as tc:
        with tc.tile_pool(name="resident", bufs=1) as res, \
             tc.tile_pool(name="psum", bufs=8, space="PSUM") as ps, \
             tc.tile_pool(name="cand", bufs=2) as candp, \
             tc.tile_pool(name="st2", bufs=2) as st2:
            xt_t = res.tile([D, NQ], mybir.dt.float32)
            nc.sync.dma_start(out=xt_t[:], in_=xt[:])
            xn_t = res.tile([D, Lp], mybir.dt.float32)
            nc.sync.dma_start(out=xn_t[:], in_=xn[:])
            nxx_t = res.tile([1, Lp], mybir.dt.float32)
            nc.sync.dma_start(out=nxx_t[:], in_=nxx[:])
            ones_t = res.tile([1, QTILE], mybir.dt.float32)
            nc.vector.memset(ones_t[:], 1.0)

            for qt in range(nqt):
                lhs = xt_t[:, qt * QTILE:(qt + 1) * QTILE]
                cand = candp.tile([QTILE, C * 8], mybir.dt.float32)
                for ci in range(C):
                    psum = ps.tile([QTILE, CHUNK], mybir.dt.float32)
                    sl = slice(ci * CHUNK, (ci + 1) * CHUNK)
                    nc.tensor.matmul(psum[:], lhsT=lhs, rhs=xn_t[:, sl],
                                     start=True, stop=False)
                    nc.tensor.matmul(psum[:], lhsT=ones_t[:], rhs=nxx_t[:, sl],
                                     start=False, stop=True)
                    nc.vector.max(out=cand[:, ci * 8:(ci + 1) * 8], in_=psum[:])

                m1 = st2.tile([QTILE, 8], mybir.dt.float32, tag="m1")
                i1 = st2.tile([QTILE, 8], mybir.dt.uint32, tag="i1")
                cand2 = candp.tile([QTILE, C * 8], mybir.dt.float32, tag="cand2")
                m2 = st2.tile([QTILE, 8], mybir.dt.float32, tag="m2")
                i2 = st2.tile([QTILE, 8], mybir.dt.uint32, tag="i2")
                nc.vector.max(out=m1[:], in_=cand[:])
                nc.vector.max_index(out=i1[:], in_max=m1[:], in_values=cand[:])
                nc.vector.match_replace(out=cand2[:], in_to_replace=m1[:],
                                        in_values=cand[:], imm_value=-3e38)
                nc.vector.max(out=m2[:], in_=cand2[:])
                nc.vector.max_index(out=i2[:], in_max=m2[:], in_values=cand2[:])

                vout = st2.tile([QTILE, K], mybir.dt.float32, tag="vout")
                iout = st2.tile([QTILE, K], mybir.dt.uint32, tag="iout")
                nc.vector.tensor_copy(vout[:, 0:8], m1[:])
                nc.vector.tensor_copy(vout[:, 8:16], m2[:])
                nc.vector.tensor_copy(iout[:, 0:8], i1[:])
                nc.vector.tensor_copy(iout[:, 8:16], i2[:])
                qsl = slice(qt * QTILE, (qt + 1) * QTILE)
                nc.sync.dma_start(out=out_val[qsl], in_=vout[:])
                nc.sync.dma_start(out=out_idx[qsl], in_=iout[:])

    nc.compile()
    return nc


def _get_program(D, NQ, Lp):
    key = (D, NQ, Lp)
    if key not in _compiled_cache:
        _compiled_cache[key] = _build_program(D, NQ, Lp)
    return _compiled_cache[key]


def prepare(X_train, y_train, X_test):
    """Host prep: shard/sort/pad; returns (nc, in_maps, chunk_label, NQ)."""
    X_train = np.ascontiguousarray(np.asarray(X_train, dtype=np.float32))
    X_test = np.ascontiguousarray(np.asarray(X_test, dtype=np.float32))
    y_np = np.asarray(y_train)
    N, D = X_train.shape
    NQ = X_test.shape[0]

    # ---- host prep: label-sort, shard, class-pure 512 padding ----
    order = np.argsort(y_np, kind="stable")
    Xs = X_train[order]
    ys = y_np[order]
    xx = np.einsum("ij,ij->i", Xs.astype(np.float64), Xs.astype(np.float64))
    xx = xx.astype(np.float32)

    per_core = N // NCORES
    assert per_core * NCORES == N
    core_segs = []
    for k in range(NCORES):
        yk = ys[k * per_core:(k + 1) * per_core]
        b = [0] + list(np.nonzero(np.diff(yk))[0] + 1) + [per_core]
        core_segs.append([(b[i], b[i + 1], int(yk[b[i]]))
                          for i in range(len(b) - 1)])

    def plen(segs):
        return sum(((e - s + CHUNK - 1) // CHUNK) * CHUNK for s, e, _ in segs)

    Lp = max(plen(s) for s in core_segs)
    C = Lp // CHUNK

    PAD_XX = np.float32(4e9)
    xnT = np.zeros((NCORES, D, Lp), np.float32)
    nxx = np.full((NCORES, 1, Lp), -PAD_XX, np.float32)
    chunk_label = np.zeros((NCORES, C), np.int64)
    for k in range(NCORES):
        lo = k * per_core
        pos = 0
        for s, e, lab in core_segs[k]:
            n = e - s
            xnT[k, :, pos:pos + n] = Xs[lo + s:lo + e].T
            nxx[k, 0, pos:pos + n] = -xx[lo + s:lo + e]
            nch = (n + CHUNK - 1) // CHUNK
            chunk_label[k, pos // CHUNK:pos // CHUNK + nch] = lab
            pos += nch * CHUNK

    xtT = np.ascontiguousarray((2.0 * X_test).T)  # [D, NQ], exact x2

    nc = _get_program(D, NQ, Lp)
    in_maps = [{"xt": xtT, "xn": np.ascontiguousarray(xnT[k]),
                "nxx": nxx[k]} for k in range(NCORES)]
    return nc, in_maps, chunk_label, NQ


def merge(results, chunk_label, NQ):
    """Host merge: 8 x 16 candidates/query -> global top-16 -> vote."""
    vals = np.stack([results[k]["out_val"] for k in range(NCORES)], axis=1)
    idxs = np.stack([results[k]["out_idx"] for k in range(NCORES)], axis=1)
    vals = vals.reshape(NQ, NCORES * K)
    labs = chunk_label[
        np.repeat(np.arange(NCORES)[None, :], NQ, axis=0).repeat(K, axis=1),
        (idxs.reshape(NQ, NCORES * K).astype(np.int64) >> 3)]
    sel = np.argpartition(-vals, K - 1, axis=1)[:, :K]
    top_lab = np.take_along_axis(labs, sel, axis=1)
    counts = np.zeros((NQ, NUM_CLASSES), np.int64)
    for c in range(NUM_CLASSES):
        counts[:, c] = (top_lab == c).sum(1)
    return counts.argmax(1).astype(np.int64)


def kernel(X_train, y_train, X_test):
    from concourse.bass_utils import run_bass_kernel_spmd
    nc, in_maps, chunk_label, NQ = prepare(X_train, y_train, X_test)
    res = run_bass_kernel_spmd(nc, in_maps, core_ids=list(range(NCORES)))
    return merge(res.results, chunk_label, NQ)
